# revision 14
# baseline (speedup 1.0000x reference)
"""Trainium2 8-core kernel for biased-attention with sigmoid gating.

Reference computation (per batch b):
  q = heads(q_x @ Wq) * C**-0.5 ; k = heads(kv_x @ Wk) ; v = heads(kv_x @ Wv)
  a = softmax(q k^T + bias1 + bias2, axis=-1)
  o = (a @ v) gated by sigmoid(q_x @ Wg + bg), then @ Wo + bo

Shapes: B=2, Q=K=2048, CQ=CK=CV=256, H=8, C=32, CO=256.

Sharding: 8 cores = 2 batches x 4 query-quarters (512 rows each). Each core
computes all 8 heads for its rows; no cross-core communication is needed.

The dominant cost is streaming the two [B,H,Q,K] bias tensors. They are
host-cast to bf16 (the score tile is truncated to bf16 before exp anyway,
exactly as the f32-bias variant did on-chip), which halves the HBM floor to
~34 MB per core (~95 us at ~360 GB/s). Every engine is budgeted under that:
  - b1 streams on the SP HWDGE ring and b2 on the GpSimd SWDGE ring,
    both with a host k-interleave that keeps every DMA descriptor a 2 KB
    contiguous run (1 KB descriptors halve the per-queue dispatch rate);
    DVE sums b1+b2 in place with one 2x-rate bf16 pass per head;
  - the PE computes QK^T (K=32, N=512) into 2-bank PSUM groups; DVE adds
    the bias sum onto each group while moving it to SBUF as bf16;
  - ScalarE applies exp once per head on a fused [128, 8192] tile, and the
    PE consumes exp(S^T) as the moving operand of the PV matmul;
  - V carries an extra all-ones column per head, so PV emits the softmax
    denominators for free; a tiny [33,128] PE back-transpose restores the
    natural orientation for the per-row normalization;
  - Q/K projections are M=128 matmuls packing 4 heads per partition tile
    (stationary row bases 0/32/64/96), halving projection PE time;
  - PE work is software-pipelined QK(h) -> PV(h-1) so it never stalls on
    the bias-add/exp latency and the HAM clock gate stays at 2.4 GHz.
"""

import numpy as np

B, Q, K, CQ, H, C, CO = 2, 2048, 2048, 256, 8, 32, 256
HC = H * C  # 256
QS = Q // 4  # 512 query rows per core
N_CORES = 8
KTILES = K // 128  # 16
SCALE = float(C) ** -0.5

_CACHED = {}


def _build():
    import concourse.bass as bass
    import concourse.mybir as mybir
    import concourse.tile as tile
    from concourse import bacc
    from concourse.masks import make_identity

    f32 = mybir.dt.float32
    bf16 = mybir.dt.bfloat16
    AF = mybir.ActivationFunctionType
    ALU = mybir.AluOpType

    nc = bacc.Bacc(None, target_bir_lowering=False)

    # activations arrive host-transposed and pre-cast to bf16: [C, rows]
    qxTd = nc.declare_dram_parameter("qxT", [CQ, QS], bf16, isOutput=False)
    kvxTd = nc.declare_dram_parameter("kvxT", [CQ, K], bf16, isOutput=False)
    # biases arrive host-transposed, bf16, k-interleaved: [h, pr, p, s, q]
    # holds bias[h, k=pr*256+2p+s, q] so (s,q) is a contiguous 2 KB run
    b1 = nc.declare_dram_parameter("b1", [H, 8, 128, 2, QS], bf16, isOutput=False)
    b2 = nc.declare_dram_parameter("b2", [H, 8, 128, 2, QS], bf16, isOutput=False)
    # weights pre-cast to bf16 on host; Wq carries the C**-0.5 scale
    Wq = nc.declare_dram_parameter("Wq", [CQ, HC], bf16, isOutput=False)
    Wk = nc.declare_dram_parameter("Wk", [CQ, HC], bf16, isOutput=False)
    Wv = nc.declare_dram_parameter("Wv", [CQ, HC], bf16, isOutput=False)
    Wg = nc.declare_dram_parameter("Wg", [CQ, HC], bf16, isOutput=False)
    bg = nc.declare_dram_parameter("bg", [HC], f32, isOutput=False)
    Wo = nc.declare_dram_parameter("Wo", [HC, CO], bf16, isOutput=False)
    bo = nc.declare_dram_parameter("bo", [CO], f32, isOutput=False)
    out = nc.declare_dram_parameter("out", [QS, CO], f32, isOutput=True)

    with tile.TileContext(nc) as tc:
        with (
            tc.tile_pool(name="singles", bufs=1) as singles,
            tc.tile_pool(name="stage", bufs=3) as stage,
            tc.tile_pool(name="bias", bufs=3) as biasp,
            tc.tile_pool(name="ew", bufs=3) as ewp,
            tc.tile_pool(name="ps", bufs=1, space="PSUM") as psp,
        ):
            ident = singles.tile([128, 128], bf16)
            make_identity(nc, ident)
            identf = singles.tile([128, 128], f32, tag="identf")
            make_identity(nc, identf)

            # ---- setup loads: projection-critical tensors go FIRST on
            # the SP ring (ahead of the b1 stream) so the PE can start within
            # ~6 us; everything else rides the Act ring. ----
            wbf = {}
            for name, w in (("Wq", Wq), ("Wk", Wk)):
                wtile = singles.tile([128, 2, 256], bf16, tag=f"w_{name}")
                nc.sync.dma_start(
                    out=wtile, in_=w[:, :].rearrange("(a p) c -> p a c", p=128)
                )
                wbf[name] = wtile
            qxT = singles.tile([128, 2, QS], bf16, tag="qxT")
            nc.sync.dma_start(
                out=qxT, in_=qxTd[:, :].rearrange("(a p) q -> p a q", p=128)
            )
            kvxT = singles.tile([128, 2, K], bf16, tag="kvxT")
            nc.sync.dma_start(
                out=kvxT, in_=kvxTd[:, :].rearrange("(a p) k -> p a k", p=128)
            )
            for name, w in (("Wv", Wv), ("Wg", Wg), ("Wo", Wo)):
                wtile = singles.tile([128, 2, 256], bf16, tag=f"w_{name}")
                nc.scalar.dma_start(
                    out=wtile, in_=w[:, :].rearrange("(a p) c -> p a c", p=128)
                )
                wbf[name] = wtile
            bg_bc = singles.tile([128, HC], f32, tag="bg")
            nc.scalar.dma_start(out=bg_bc, in_=bg[:].partition_broadcast(128))
            bo_bc = singles.tile([128, CO], f32, tag="bo")
            nc.scalar.dma_start(out=bo_bc, in_=bo[:].partition_broadcast(128))

            # ---- bias stream: b1 on the SP HWDGE ring, b2 on the
            # GpSimd SWDGE ring; one 1 MB DMA per head per bias. The host
            # interleaves k as k = pr*256 + 2p + s so each descriptor moves
            # a contiguous 2 KB (s,q) run; on-chip row p of pair pr holds
            # k-tiles kt=2pr+s, matching the k-permuted kvxT columns. ----
            bs_tiles = []
            for h in range(H):
                halves = []
                for z in range(2):
                    Bt = biasp.tile([128, 4, 2, 512], bf16, tag="b1", bufs=8)
                    sl = slice(z * 4, (z + 1) * 4)
                    nc.sync.dma_start(
                        out=Bt, in_=b1[h, sl].rearrange("pr p s q -> p pr s q")
                    )
                    nc.gpsimd.dma_start(
                        out=Bt,
                        in_=b2[h, sl].rearrange("pr p s q -> p pr s q"),
                        accum_op=ALU.add,
                    )
                    halves.append(Bt)
                bs_tiles.append(halves)

            # ---- projections: 4 heads packed per 128-partition tile
            # (head h at partition base (h%4)*32 of half j=h//4), produced
            # by full-width M=128 matmuls. ----
            KT = singles.tile([128, 2, K], bf16, tag="KT")
            QT = singles.tile([128, 2, QS], bf16, tag="QT")
            Vn = singles.tile([128, KTILES, H * 33], bf16, tag="Vn")
            nc.vector.memset(Vn, 1.0)
            Gn = singles.tile([128, 4, HC], f32, tag="Gn")
            O_all = singles.tile([128, 4, HC], f32, tag="O_all")
            oTs = singles.tile([33, H, QS], f32, tag="oTs")

            def hsl(h):
                return slice((h % 4) * 32, (h % 4) * 32 + 32)

            def proj_q(j):
                ps = psp.tile([128, 2, 512, 1], f32, tag="scores", bufs=3)
                for ck in range(2):
                    nc.tensor.matmul(
                        ps[:, 0, :, 0],
                        wbf["Wq"][:, ck, j * 128:(j + 1) * 128],
                        qxT[:, ck, :],
                        start=(ck == 0),
                        stop=(ck == 1),
                    )
                nc.scalar.copy(QT[:, j, :], ps[:, 0, :, 0])

            def proj_k(j):
                # K columns j*128..(j+1)*128 = heads 4j..4j+3
                for kc in range(4):
                    ps = psp.tile([128, 2, 512, 1], f32, tag="scores", bufs=3)
                    for ck in range(2):
                        nc.tensor.matmul(
                            ps[:, 0, :, 0],
                            wbf["Wk"][:, ck, j * 128:(j + 1) * 128],
                            kvxT[:, ck, kc * 512:(kc + 1) * 512],
                            start=(ck == 0),
                            stop=(ck == 1),
                        )
                    nc.scalar.copy(
                        KT[:, j, kc * 512:(kc + 1) * 512], ps[:, 0, :, 0]
                    )

            def proj_v():
                # V natural [128k, 16kt, 8h*33]; per head 32 V columns plus
                # an all-ones column so PV emits softmax denominators free.
                for kt in range(KTILES):
                    ps = psp.tile([128, 2, 512, 1], f32, tag="scores", bufs=3)
                    for ck in range(2):
                        nc.tensor.matmul(
                            ps[:, 0, :HC, 0],
                            kvxT[:, ck, kt * 128:(kt + 1) * 128],
                            wbf["Wv"][:, ck, :],
                            start=(ck == 0),
                            stop=(ck == 1),
                        )
                    nc.scalar.copy(
                        Vn[:, kt, :].rearrange("p (h c) -> p h c", h=H)[:, :, :32],
                        ps[:, 0, :HC, 0].rearrange("p (h c) -> p h c", h=H),
                    )

            def proj_g():
                # G natural [128q, 4qt, 256hc] f32 = sigmoid(qx @ Wg + bg)
                for qt in range(4):
                    ps = psp.tile([128, 2, 512, 1], f32, tag="scores", bufs=3)
                    for ck in range(2):
                        nc.tensor.matmul(
                            ps[:, 0, :HC, 0],
                            qxT[:, ck, qt * 128:(qt + 1) * 128],
                            wbf["Wg"][:, ck, :],
                            start=(ck == 0),
                            stop=(ck == 1),
                        )
                    gt = stage.tile([128, HC], f32, tag="gtmp")
                    nc.vector.tensor_add(gt, ps[:, 0, :HC, 0], bg_bc)
                    nc.scalar.activation(Gn[:, qt, :], gt, AF.Sigmoid)

            # ---- main attention, software-pipelined on the PE:
            # QK(h) ... PV(h-1), so PV never waits on the bias-add/exp. ----
            def qk_block(h):
                j, rows = h // 4, hsl(h)
                et = ewp.tile([128, H, 2, 512], bf16, tag="et")
                for pr in range(8):
                    Bt = bs_tiles[h][pr // 4]
                    ps = psp.tile([128, 2, 512, 1], f32, tag="scores", bufs=3)
                    for s in range(2):
                        kt = pr * 2 + s
                        nc.tensor.matmul(
                            ps[:, s, :, 0],
                            KT[rows, j, kt * 128:(kt + 1) * 128],
                            QT[rows, j, :],
                            start=True,
                            stop=True,
                            tile_position=((h % 4) * 32, 0),
                        )
                    nc.vector.tensor_tensor(
                        et[:, pr], ps[:, :, :, 0], Bt[:, pr % 4], ALU.add
                    )
                nc.scalar.activation(et, et, AF.Exp)
                return et

            def pv_block(h, et):
                o_ps = psp.tile([33, QS, 1], f32, tag="o_acc", bufs=2)
                for kt in range(KTILES):
                    nc.tensor.matmul(
                        o_ps[:, :, 0],
                        Vn[:, kt, h * 33:h * 33 + 33],
                        et[:, kt // 2, kt % 2, :],
                        start=(kt == 0),
                        stop=(kt == KTILES - 1),
                    )
                pv_tail(h, o_ps)

            def pv_tail(h, o_ps):
                nc.vector.tensor_copy(oTs[:, h, :], o_ps[:, :, 0])
                for qt in range(4):
                    on_ps = psp.tile([128, 2, 512, 1], f32, tag="scores", bufs=3)
                    nc.tensor.transpose(
                        on_ps[:, 0, :33, 0],
                        oTs[:, h, qt * 128:(qt + 1) * 128],
                        identf[:33, :33],
                    )
                    rinv = stage.tile([128, 1], f32, tag="rinv")
                    nc.vector.reciprocal(rinv, on_ps[:, 0, 32:33, 0])
                    nc.scalar.mul(
                        O_all[:, qt, h * 32:(h + 1) * 32], on_ps[:, 0, :32, 0], rinv
                    )

            proj_q(0)
            proj_k(0)
            prev = (0, qk_block(0))
            proj_v()
            proj_q(1)
            proj_k(1)
            for h in range(1, H):
                et = qk_block(h)
                pv_block(*prev)
                prev = (h, et)
            pv_block(*prev)
            proj_g()

            # ---- gating + output projection ----
            for qt in range(4):
                og = stage.tile([128, HC], f32, tag="og")
                nc.gpsimd.tensor_tensor(
                    og, O_all[:, qt, :], Gn[:, qt, :], ALU.mult
                )
                ogt_ps = psp.tile([128, 2, 512, 1], f32, tag="scores", bufs=3)
                for hcc in range(2):
                    nc.tensor.transpose(
                        ogt_ps[:, hcc, :128, 0],
                        og[:, hcc * 128:(hcc + 1) * 128],
                        identf,
                    )
                ogt = stage.tile([128, 2, 128], bf16, tag="ogt")
                nc.vector.tensor_copy(ogt, ogt_ps[:, :, :128, 0])
                f_ps = psp.tile([128, 2, 512, 1], f32, tag="scores", bufs=3)
                for hcc in range(2):
                    nc.tensor.matmul(
                        f_ps[:, 0, :CO, 0],
                        ogt[:, hcc, :],
                        wbf["Wo"][:, hcc, :],
                        start=(hcc == 0),
                        stop=(hcc == 1),
                    )
                o_sb = stage.tile([128, CO], f32, tag="o_out")
                nc.vector.tensor_add(o_sb, f_ps[:, 0, :CO, 0], bo_bc)
                nc.scalar.dma_start(out=out[qt * 128:(qt + 1) * 128, :], in_=o_sb)

    nc.compile()
    return nc


def _get_nc():
    if "nc" not in _CACHED:
        _CACHED["nc"] = _build()
    return _CACHED["nc"]


def kernel(**inputs):
    from concourse.bass_utils import run_bass_kernel_spmd

    import ml_dtypes

    bf = ml_dtypes.bfloat16
    nc = _get_nc()
    inp = {k: np.asarray(v, dtype=np.float32) for k, v in inputs.items()}
    wq_b = (inp["Wq"] * SCALE).astype(bf)
    wk_b = inp["Wk"].astype(bf)
    wv_b = inp["Wv"].astype(bf)
    wg_b = inp["Wg"].astype(bf)
    wo_b = inp["Wo"].astype(bf)
    # on-chip k order is m = kt*128 + p with k = (kt//2)*256 + 2p + (kt%2);
    # permute kvxT columns so every on-chip consumer stays contiguous
    m = np.arange(K)
    kmap = (m // 256) * 256 + 2 * (m % 128) + (m // 128) % 2
    in_maps = []
    for c in range(N_CORES):
        b, qi = c // 4, c % 4
        q0 = qi * QS
        in_maps.append({
            "qxT": np.ascontiguousarray(inp["q_x"][b, q0:q0 + QS, :].T).astype(bf),
            "kvxT": np.ascontiguousarray(inp["kv_x"][b].T[:, kmap]).astype(bf),
            "b1": np.ascontiguousarray(
                inp["bias1"][b, :, q0:q0 + QS, :].transpose(0, 2, 1)
            ).astype(bf).reshape(H, 8, 128, 2, QS),
            "b2": np.ascontiguousarray(
                inp["bias2"][b, :, q0:q0 + QS, :].transpose(0, 2, 1)
            ).astype(bf).reshape(H, 8, 128, 2, QS),
            "Wq": wq_b, "Wk": wk_b, "Wv": wv_b, "Wg": wg_b,
            "bg": inp["bg"], "Wo": wo_b, "bo": inp["bo"],
        })
    res = run_bass_kernel_spmd(nc, in_maps, core_ids=list(range(N_CORES)))
    outa = np.empty((B, Q, CO), np.float32)
    for c in range(N_CORES):
        b, qi = c // 4, c % 4
        outa[b, qi * QS:(qi + 1) * QS, :] = res.results[c]["out"]
    return outa


# revision 15
# speedup vs baseline: 1.3370x; 1.3370x over previous
"""Trainium2 8-core kernel for biased-attention with sigmoid gating.

Reference computation (per batch b):
  q = heads(q_x @ Wq) * C**-0.5 ; k = heads(kv_x @ Wk) ; v = heads(kv_x @ Wv)
  a = softmax(q k^T + bias1 + bias2, axis=-1)
  o = (a @ v) gated by sigmoid(q_x @ Wg + bg), then @ Wo + bo

Shapes: B=2, Q=K=2048, CQ=CK=CV=256, H=8, C=32, CO=256.

Sharding: 8 cores = 2 batches x 4 query-quarters (512 rows each). Each core
computes all 8 heads for its rows; no cross-core communication is needed.

The dominant cost is streaming the two [B,H,Q,K] bias tensors. They are
host-cast to bf16 (the score tile is truncated to bf16 before exp anyway,
exactly as the f32-bias variant did on-chip), which halves the HBM floor to
~34 MB per core. Every engine is budgeted under that:
  - b1 streams on the SP HWDGE ring; b2 streams as GpSimd-SWDGE DMAs with
    accum_op=add, so the DMA compute engines produce b1+b2 in SBUF and no
    compute engine spends a pass on the bias sum;
  - the PE computes QK^T (K=32, N=512) into 2-bank PSUM groups; DVE adds
    the bias sum onto each group while moving it to SBUF as bf16;
  - ScalarE applies exp once per head on a fused [128, 8192] tile, and the
    PE consumes exp(S^T) as the moving operand of the PV matmul;
  - V carries an extra all-ones column per head, so PV emits the softmax
    denominators for free; a tiny [33,128] PE back-transpose restores the
    natural orientation for the per-row normalization;
  - Q/K projections are M=128 matmuls packing 4 heads per partition tile
    (stationary row bases 0/32/64/96), halving projection PE time;
  - PE work is software-pipelined QK(h) -> PV(h-1) so the PV block never
    waits on the bias-add/exp latency of its own head.
"""

import numpy as np

B, Q, K, CQ, H, C, CO = 2, 2048, 2048, 256, 8, 32, 256
HC = H * C  # 256
QS = Q // 4  # 512 query rows per core
N_CORES = 8
KTILES = K // 128  # 16
SCALE = float(C) ** -0.5

_CACHED = {}


def _build():
    import concourse.bass as bass
    import concourse.mybir as mybir
    import concourse.tile as tile
    from concourse import bacc
    from concourse.masks import make_identity

    f32 = mybir.dt.float32
    bf16 = mybir.dt.bfloat16
    AF = mybir.ActivationFunctionType
    ALU = mybir.AluOpType

    nc = bacc.Bacc(None, target_bir_lowering=False)

    # activations arrive host-transposed and pre-cast to bf16: [C, rows]
    qxTd = nc.declare_dram_parameter("qxT", [CQ, QS], bf16, isOutput=False)
    kvxTd = nc.declare_dram_parameter("kvxT", [CQ, K], bf16, isOutput=False)
    # biases arrive host-transposed [H, K, QS] (k-major) and pre-cast to bf16
    b1 = nc.declare_dram_parameter("b1", [H, K, QS], bf16, isOutput=False)
    b2 = nc.declare_dram_parameter("b2", [H, K, QS], bf16, isOutput=False)
    # weights pre-cast to bf16 on host; Wq carries the C**-0.5 scale
    Wq = nc.declare_dram_parameter("Wq", [CQ, HC], bf16, isOutput=False)
    Wk = nc.declare_dram_parameter("Wk", [CQ, HC], bf16, isOutput=False)
    Wv = nc.declare_dram_parameter("Wv", [CQ, HC], bf16, isOutput=False)
    Wg = nc.declare_dram_parameter("Wg", [CQ, HC], bf16, isOutput=False)
    bg = nc.declare_dram_parameter("bg", [HC], f32, isOutput=False)
    Wo = nc.declare_dram_parameter("Wo", [HC, CO], bf16, isOutput=False)
    bo = nc.declare_dram_parameter("bo", [CO], f32, isOutput=False)
    out = nc.declare_dram_parameter("out", [QS, CO], f32, isOutput=True)

    with tile.TileContext(nc) as tc:
        with (
            tc.tile_pool(name="singles", bufs=1) as singles,
            tc.tile_pool(name="stage", bufs=3) as stage,
            tc.tile_pool(name="bias", bufs=4) as biasp,
            tc.tile_pool(name="tw", bufs=2) as twp,
            tc.tile_pool(name="ew", bufs=2) as ewp,
            tc.tile_pool(name="ps", bufs=1, space="PSUM") as psp,
        ):
            ident = singles.tile([128, 128], bf16)
            make_identity(nc, ident)
            identf = singles.tile([128, 128], f32, tag="identf")
            make_identity(nc, identf)

            # ---- setup loads on the scalar (Act) HWDGE ring; the SP ring
            # is reserved for the b1 stream, which starts at t=0 ----
            kvxT = singles.tile([128, 2, K], bf16, tag="kvxT")
            nc.scalar.dma_start(
                out=kvxT, in_=kvxTd[:, :].rearrange("(a p) k -> p a k", p=128)
            )
            wbf = {}
            for name, w in (("Wk", Wk), ("Wq", Wq)):
                wtile = singles.tile([128, 2, 256], bf16, tag=f"w_{name}")
                nc.scalar.dma_start(
                    out=wtile, in_=w[:, :].rearrange("(a p) c -> p a c", p=128)
                )
                wbf[name] = wtile
            qxT = singles.tile([128, 2, QS], bf16, tag="qxT")
            nc.scalar.dma_start(
                out=qxT, in_=qxTd[:, :].rearrange("(a p) q -> p a q", p=128)
            )
            for name, w in (("Wv", Wv), ("Wg", Wg), ("Wo", Wo)):
                wtile = singles.tile([128, 2, 256], bf16, tag=f"w_{name}")
                nc.scalar.dma_start(
                    out=wtile, in_=w[:, :].rearrange("(a p) c -> p a c", p=128)
                )
                wbf[name] = wtile
            bg_bc = singles.tile([128, HC], f32, tag="bg")
            nc.scalar.dma_start(out=bg_bc, in_=bg[:].partition_broadcast(128))
            bo_bc = singles.tile([128, CO], f32, tag="bo")
            nc.scalar.dma_start(out=bo_bc, in_=bo[:].partition_broadcast(128))

            # ---- bias stream: b1 on the SP ring, b2 as SWDGE accum-DMAs
            # that add onto b1's tile, yielding bsum = b1+b2 in SBUF with no
            # compute-engine pass. One 1 MB DMA per head per bias. Layout
            # [128p, 8pr, 2s, 512q] with k = (2*pr+s)*128 + p, so k-tile
            # kt=(2*pr+s) is the (pr, s) column slice. ----
            bs_tiles = []
            for h in range(H):
                Bt = biasp.tile([128, H, 2, 512], bf16, tag="bsum")
                nc.sync.dma_start(
                    out=Bt,
                    in_=b1[h, :, :].rearrange("(pr s p) q -> p pr s q", p=128, s=2),
                )
                nc.gpsimd.dma_start(
                    out=Bt,
                    in_=b2[h, :, :].rearrange("(pr s p) q -> p pr s q", p=128, s=2),
                    accum_op=ALU.add,
                )
                bs_tiles.append(Bt)

            # ---- projections: 4 heads packed per 128-partition tile
            # (head h at partition base (h%4)*32 of half j=h//4), produced
            # by full-width M=128 matmuls. ----
            KT = singles.tile([128, 2, K], bf16, tag="KT")
            QT = singles.tile([128, 2, QS], bf16, tag="QT")
            Vn = singles.tile([128, KTILES, H * 33], bf16, tag="Vn")
            nc.vector.memset(Vn, 1.0)
            Gn = singles.tile([128, 4, HC], f32, tag="Gn")
            O_all = singles.tile([128, 4, HC], f32, tag="O_all")
            oTs = singles.tile([33, H, QS], f32, tag="oTs")

            def hsl(h):
                return slice((h % 4) * 32, (h % 4) * 32 + 32)

            def proj_kq(j):
                # K/Q columns j*128..(j+1)*128 = heads 4j..4j+3
                for kc in range(4):
                    ps = psp.tile([128, 2, 512, 1], f32, tag="scores", bufs=2)
                    for ck in range(2):
                        nc.tensor.matmul(
                            ps[:, 0, :, 0],
                            wbf["Wk"][:, ck, j * 128:(j + 1) * 128],
                            kvxT[:, ck, kc * 512:(kc + 1) * 512],
                            start=(ck == 0),
                            stop=(ck == 1),
                        )
                    nc.scalar.copy(
                        KT[:, j, kc * 512:(kc + 1) * 512], ps[:, 0, :, 0]
                    )
                ps = psp.tile([128, 2, 512, 1], f32, tag="scores", bufs=2)
                for ck in range(2):
                    nc.tensor.matmul(
                        ps[:, 0, :, 0],
                        wbf["Wq"][:, ck, j * 128:(j + 1) * 128],
                        qxT[:, ck, :],
                        start=(ck == 0),
                        stop=(ck == 1),
                    )
                nc.scalar.copy(QT[:, j, :], ps[:, 0, :, 0])

            def proj_v():
                # V natural [128k, 16kt, 8h*33]; per head 32 V columns plus
                # an all-ones column so PV emits softmax denominators free.
                for kt in range(KTILES):
                    ps = psp.tile([128, 2, 512, 1], f32, tag="scores", bufs=2)
                    for ck in range(2):
                        nc.tensor.matmul(
                            ps[:, 0, :HC, 0],
                            kvxT[:, ck, kt * 128:(kt + 1) * 128],
                            wbf["Wv"][:, ck, :],
                            start=(ck == 0),
                            stop=(ck == 1),
                        )
                    nc.scalar.copy(
                        Vn[:, kt, :].rearrange("p (h c) -> p h c", h=H)[:, :, :32],
                        ps[:, 0, :HC, 0].rearrange("p (h c) -> p h c", h=H),
                    )

            def proj_g():
                # G natural [128q, 4qt, 256hc] f32 = sigmoid(qx @ Wg + bg)
                for qt in range(4):
                    ps = psp.tile([128, 2, 512, 1], f32, tag="scores", bufs=2)
                    for ck in range(2):
                        nc.tensor.matmul(
                            ps[:, 0, :HC, 0],
                            qxT[:, ck, qt * 128:(qt + 1) * 128],
                            wbf["Wg"][:, ck, :],
                            start=(ck == 0),
                            stop=(ck == 1),
                        )
                    gt = stage.tile([128, HC], f32, tag="gtmp")
                    nc.vector.tensor_add(gt, ps[:, 0, :HC, 0], bg_bc)
                    nc.scalar.activation(Gn[:, qt, :], gt, AF.Sigmoid)

            # ---- main attention, software-pipelined on the PE:
            # QK(h) ... PV(h-1), so PV never waits on the bias-add/exp. ----
            def qk_block(h):
                j, rows = h // 4, hsl(h)
                tt = twp.tile([128, H, 2, 512], bf16, tag="t")
                et = ewp.tile([128, H, 2, 512], bf16, tag="et")
                Bt = bs_tiles[h]
                for pr in range(8):
                    ps = psp.tile([128, 2, 512, 1], f32, tag="scores", bufs=2)
                    for s in range(2):
                        kt = pr * 2 + s
                        nc.tensor.matmul(
                            ps[:, s, :, 0],
                            KT[rows, j, kt * 128:(kt + 1) * 128],
                            QT[rows, j, :],
                            start=True,
                            stop=True,
                            tile_position=((h % 4) * 32, 0),
                        )
                    nc.vector.tensor_tensor(
                        tt[:, pr], ps[:, :, :, 0], Bt[:, pr], ALU.add
                    )
                nc.scalar.activation(et, tt, AF.Exp)
                return et

            def pv_block(h, et):
                hcol = h * 33
                o_ps = psp.tile([33, QS, 1], f32, tag="o_acc", bufs=2)
                for kt in range(KTILES):
                    nc.tensor.matmul(
                        o_ps[:, :, 0],
                        Vn[:, kt, hcol:hcol + 33],
                        et[:, kt // 2, kt % 2, :],
                        start=(kt == 0),
                        stop=(kt == KTILES - 1),
                    )
                nc.vector.tensor_copy(oTs[:, h, :], o_ps[:, :, 0])
                for qt in range(4):
                    on_ps = psp.tile([128, 33, 1], f32, tag="onat", bufs=1)
                    nc.tensor.transpose(
                        on_ps[:, :, 0],
                        oTs[:, h, qt * 128:(qt + 1) * 128],
                        identf[:33, :33],
                    )
                    rinv = stage.tile([128, 1], f32, tag="rinv")
                    nc.vector.reciprocal(rinv, on_ps[:, 32:33, 0])
                    nc.vector.tensor_scalar_mul(
                        O_all[:, qt, h * 32:(h + 1) * 32], on_ps[:, :32, 0], rinv
                    )

            proj_kq(0)
            proj_g()
            prev = (0, qk_block(0))
            proj_v()
            proj_kq(1)
            for h in range(1, H):
                et = qk_block(h)
                pv_block(*prev)
                prev = (h, et)
            pv_block(*prev)

            # ---- gating + output projection ----
            for qt in range(4):
                og = stage.tile([128, HC], bf16, tag="og")
                nc.gpsimd.tensor_tensor(
                    og, O_all[:, qt, :], Gn[:, qt, :], ALU.mult
                )
                ogt_ps = psp.tile([128, 2, 128], bf16, tag="et_ps", bufs=1)
                for hcc in range(2):
                    nc.tensor.transpose(
                        ogt_ps[:, hcc, :], og[:, hcc * 128:(hcc + 1) * 128], ident
                    )
                ogt = stage.tile([128, 2, 128], bf16, tag="ogt")
                nc.vector.tensor_copy(ogt, ogt_ps)
                f_ps = psp.tile([128, 2, 512, 1], f32, tag="scores", bufs=2)
                for hcc in range(2):
                    nc.tensor.matmul(
                        f_ps[:, 0, :CO, 0],
                        ogt[:, hcc, :],
                        wbf["Wo"][:, hcc, :],
                        start=(hcc == 0),
                        stop=(hcc == 1),
                    )
                o_sb = stage.tile([128, CO], f32, tag="o_out")
                nc.vector.tensor_add(o_sb, f_ps[:, 0, :CO, 0], bo_bc)
                nc.scalar.dma_start(out=out[qt * 128:(qt + 1) * 128, :], in_=o_sb)

    nc.compile()
    return nc


def _get_nc():
    if "nc" not in _CACHED:
        _CACHED["nc"] = _build()
    return _CACHED["nc"]


def kernel(**inputs):
    from concourse.bass_utils import run_bass_kernel_spmd

    import ml_dtypes

    bf = ml_dtypes.bfloat16
    nc = _get_nc()
    inp = {k: np.asarray(v, dtype=np.float32) for k, v in inputs.items()}
    wq_b = (inp["Wq"] * SCALE).astype(bf)
    wk_b = inp["Wk"].astype(bf)
    wv_b = inp["Wv"].astype(bf)
    wg_b = inp["Wg"].astype(bf)
    wo_b = inp["Wo"].astype(bf)
    in_maps = []
    for c in range(N_CORES):
        b, qi = c // 4, c % 4
        q0 = qi * QS
        in_maps.append({
            "qxT": np.ascontiguousarray(inp["q_x"][b, q0:q0 + QS, :].T).astype(bf),
            "kvxT": np.ascontiguousarray(inp["kv_x"][b].T).astype(bf),
            "b1": np.ascontiguousarray(
                inp["bias1"][b, :, q0:q0 + QS, :].transpose(0, 2, 1)
            ).astype(bf),
            "b2": np.ascontiguousarray(
                inp["bias2"][b, :, q0:q0 + QS, :].transpose(0, 2, 1)
            ).astype(bf),
            "Wq": wq_b, "Wk": wk_b, "Wv": wv_b, "Wg": wg_b,
            "bg": inp["bg"], "Wo": wo_b, "bo": inp["bo"],
        })
    res = run_bass_kernel_spmd(nc, in_maps, core_ids=list(range(N_CORES)))
    outa = np.empty((B, Q, CO), np.float32)
    for c in range(N_CORES):
        b, qi = c // 4, c % 4
        outa[b, qi * QS:(qi + 1) * QS, :] = res.results[c]["out"]
    return outa


# revision 16
# speedup vs baseline: 1.3506x; 1.0101x over previous
"""Trainium2 8-core kernel for biased-attention with sigmoid gating.

Reference computation (per batch b):
  q = heads(q_x @ Wq) * C**-0.5 ; k = heads(kv_x @ Wk) ; v = heads(kv_x @ Wv)
  a = softmax(q k^T + bias1 + bias2, axis=-1)
  o = (a @ v) gated by sigmoid(q_x @ Wg + bg), then @ Wo + bo

Shapes: B=2, Q=K=2048, CQ=CK=CV=256, H=8, C=32, CO=256.

Sharding: 8 cores = 2 batches x 4 query-quarters (512 rows each). Each core
computes all 8 heads for its rows; no cross-core communication is needed.

The dominant cost is streaming the two [B,H,Q,K] bias tensors. They are
host-cast to bf16 (the score tile is truncated to bf16 before exp anyway,
exactly as the f32-bias variant did on-chip), which halves the HBM floor to
~34 MB per core. Every engine is budgeted under that:
  - b1 streams on the SP HWDGE ring; b2 streams as GpSimd-SWDGE DMAs with
    accum_op=add, so the DMA compute engines produce b1+b2 in SBUF and no
    compute engine spends a pass on the bias sum;
  - the PE computes QK^T (K=32, N=512) into 2-bank PSUM groups; DVE adds
    the bias sum onto each group while moving it to SBUF as bf16;
  - ScalarE applies exp once per head on a fused [128, 8192] tile, and the
    PE consumes exp(S^T) as the moving operand of the PV matmul;
  - V carries an extra all-ones column per head, so PV emits the softmax
    denominators for free; a tiny [33,128] PE back-transpose restores the
    natural orientation for the per-row normalization;
  - Q/K projections are M=128 matmuls packing 4 heads per partition tile
    (stationary row bases 0/32/64/96), halving projection PE time;
  - PE work is software-pipelined QK(h) -> PV(h-1) so the PV block never
    waits on the bias-add/exp latency of its own head.
"""

import numpy as np

B, Q, K, CQ, H, C, CO = 2, 2048, 2048, 256, 8, 32, 256
HC = H * C  # 256
QS = Q // 4  # 512 query rows per core
N_CORES = 8
KTILES = K // 128  # 16
SCALE = float(C) ** -0.5

_CACHED = {}


def _build():
    import concourse.bass as bass
    import concourse.mybir as mybir
    import concourse.tile as tile
    from concourse import bacc
    from concourse.masks import make_identity

    f32 = mybir.dt.float32
    bf16 = mybir.dt.bfloat16
    AF = mybir.ActivationFunctionType
    ALU = mybir.AluOpType

    nc = bacc.Bacc(None, target_bir_lowering=False)

    # activations arrive host-transposed and pre-cast to bf16: [C, rows]
    qxTd = nc.declare_dram_parameter("qxT", [CQ, QS], bf16, isOutput=False)
    kvxTd = nc.declare_dram_parameter("kvxT", [CQ, K], bf16, isOutput=False)
    # biases arrive host-transposed, bf16, k-interleaved: [h, pr, p, s, q]
    # holds bias[h, k=pr*256+2p+s, q] so (s,q) is a contiguous 2 KB DMA run
    b1 = nc.declare_dram_parameter("b1", [H, 8, 128, 2, QS], bf16, isOutput=False)
    b2 = nc.declare_dram_parameter("b2", [H, 8, 128, 2, QS], bf16, isOutput=False)
    # weights pre-cast to bf16 on host; Wq carries the C**-0.5 scale
    Wq = nc.declare_dram_parameter("Wq", [CQ, HC], bf16, isOutput=False)
    Wk = nc.declare_dram_parameter("Wk", [CQ, HC], bf16, isOutput=False)
    Wv = nc.declare_dram_parameter("Wv", [CQ, HC], bf16, isOutput=False)
    Wg = nc.declare_dram_parameter("Wg", [CQ, HC], bf16, isOutput=False)
    bg = nc.declare_dram_parameter("bg", [HC], f32, isOutput=False)
    Wo = nc.declare_dram_parameter("Wo", [HC, CO], bf16, isOutput=False)
    bo = nc.declare_dram_parameter("bo", [CO], f32, isOutput=False)
    out = nc.declare_dram_parameter("out", [QS, CO], f32, isOutput=True)

    with tile.TileContext(nc) as tc:
        with (
            tc.tile_pool(name="singles", bufs=1) as singles,
            tc.tile_pool(name="stage", bufs=3) as stage,
            tc.tile_pool(name="bias", bufs=4) as biasp,
            tc.tile_pool(name="tw", bufs=2) as twp,
            tc.tile_pool(name="ew", bufs=2) as ewp,
            tc.tile_pool(name="ps", bufs=1, space="PSUM") as psp,
        ):
            ident = singles.tile([128, 128], bf16)
            make_identity(nc, ident)
            identf = singles.tile([128, 128], f32, tag="identf")
            make_identity(nc, identf)

            # ---- setup loads on the scalar (Act) HWDGE ring; the SP ring
            # is reserved for the b1 stream, which starts at t=0 ----
            kvxT = singles.tile([128, 2, K], bf16, tag="kvxT")
            nc.scalar.dma_start(
                out=kvxT, in_=kvxTd[:, :].rearrange("(a p) k -> p a k", p=128)
            )
            wbf = {}
            for name, w in (("Wk", Wk), ("Wq", Wq)):
                wtile = singles.tile([128, 2, 256], bf16, tag=f"w_{name}")
                nc.scalar.dma_start(
                    out=wtile, in_=w[:, :].rearrange("(a p) c -> p a c", p=128)
                )
                wbf[name] = wtile
            qxT = singles.tile([128, 2, QS], bf16, tag="qxT")
            nc.scalar.dma_start(
                out=qxT, in_=qxTd[:, :].rearrange("(a p) q -> p a q", p=128)
            )
            for name, w in (("Wv", Wv), ("Wg", Wg), ("Wo", Wo)):
                wtile = singles.tile([128, 2, 256], bf16, tag=f"w_{name}")
                nc.scalar.dma_start(
                    out=wtile, in_=w[:, :].rearrange("(a p) c -> p a c", p=128)
                )
                wbf[name] = wtile
            bg_bc = singles.tile([128, HC], f32, tag="bg")
            nc.scalar.dma_start(out=bg_bc, in_=bg[:].partition_broadcast(128))
            bo_bc = singles.tile([128, CO], f32, tag="bo")
            nc.scalar.dma_start(out=bo_bc, in_=bo[:].partition_broadcast(128))

            # ---- bias stream: b1 on the SP ring, b2 as SWDGE accum-DMAs
            # that add onto b1's tile, yielding bsum = b1+b2 in SBUF with no
            # compute-engine pass. One 1 MB DMA per head per bias. Layout
            # [128p, 8pr, 2s, 512q] with k = (2*pr+s)*128 + p, so k-tile
            # kt=(2*pr+s) is the (pr, s) column slice. ----
            # Per head: b1 rides the SP ring; b2's first half is an
            # SWDGE accum-DMA straight onto Bt (ready without compute),
            # b2's second half is a plain DMA that DVE adds in one small
            # pass — halving the costly read-modify-write accum bus time.
            bs_tiles = []
            for h in range(H):
                Bt = biasp.tile([128, H, 2, 512], bf16, tag="bsum", bufs=3)
                nc.sync.dma_start(
                    out=Bt, in_=b1[h].rearrange("pr p s q -> p pr s q")
                )
                nc.gpsimd.dma_start(
                    out=Bt[:, 0:4],
                    in_=b2[h, 0:4].rearrange("pr p s q -> p pr s q"),
                    accum_op=ALU.add,
                )
                B2t = biasp.tile([128, 4, 2, 512], bf16, tag="b2", bufs=2)
                nc.sync.dma_start(
                    out=B2t, in_=b2[h, 4:8].rearrange("pr p s q -> p pr s q")
                )
                bs_tiles.append((Bt, B2t))

            # ---- projections: 4 heads packed per 128-partition tile
            # (head h at partition base (h%4)*32 of half j=h//4), produced
            # by full-width M=128 matmuls. ----
            KT = singles.tile([128, 2, K], bf16, tag="KT")
            QT = singles.tile([128, 2, QS], bf16, tag="QT")
            Vn = singles.tile([128, KTILES, H * 33], bf16, tag="Vn")
            nc.vector.memset(Vn, 1.0)
            Gn = singles.tile([128, 4, HC], f32, tag="Gn")
            O_all = singles.tile([128, 4, HC], f32, tag="O_all")
            oTs = singles.tile([33, H, QS], f32, tag="oTs")

            def hsl(h):
                return slice((h % 4) * 32, (h % 4) * 32 + 32)

            def proj_kq(j):
                # K/Q columns j*128..(j+1)*128 = heads 4j..4j+3
                for kc in range(4):
                    ps = psp.tile([128, 2, 512, 1], f32, tag="scores", bufs=2)
                    for ck in range(2):
                        nc.tensor.matmul(
                            ps[:, 0, :, 0],
                            wbf["Wk"][:, ck, j * 128:(j + 1) * 128],
                            kvxT[:, ck, kc * 512:(kc + 1) * 512],
                            start=(ck == 0),
                            stop=(ck == 1),
                        )
                    nc.scalar.copy(
                        KT[:, j, kc * 512:(kc + 1) * 512], ps[:, 0, :, 0]
                    )
                ps = psp.tile([128, 2, 512, 1], f32, tag="scores", bufs=2)
                for ck in range(2):
                    nc.tensor.matmul(
                        ps[:, 0, :, 0],
                        wbf["Wq"][:, ck, j * 128:(j + 1) * 128],
                        qxT[:, ck, :],
                        start=(ck == 0),
                        stop=(ck == 1),
                    )
                nc.scalar.copy(QT[:, j, :], ps[:, 0, :, 0])

            def proj_v():
                # V natural [128k, 16kt, 8h*33]; per head 32 V columns plus
                # an all-ones column so PV emits softmax denominators free.
                for kt in range(KTILES):
                    ps = psp.tile([128, 2, 512, 1], f32, tag="scores", bufs=2)
                    for ck in range(2):
                        nc.tensor.matmul(
                            ps[:, 0, :HC, 0],
                            kvxT[:, ck, kt * 128:(kt + 1) * 128],
                            wbf["Wv"][:, ck, :],
                            start=(ck == 0),
                            stop=(ck == 1),
                        )
                    nc.scalar.copy(
                        Vn[:, kt, :].rearrange("p (h c) -> p h c", h=H)[:, :, :32],
                        ps[:, 0, :HC, 0].rearrange("p (h c) -> p h c", h=H),
                    )

            def proj_g():
                # G natural [128q, 4qt, 256hc] f32 = sigmoid(qx @ Wg + bg)
                for qt in range(4):
                    ps = psp.tile([128, 2, 512, 1], f32, tag="scores", bufs=2)
                    for ck in range(2):
                        nc.tensor.matmul(
                            ps[:, 0, :HC, 0],
                            qxT[:, ck, qt * 128:(qt + 1) * 128],
                            wbf["Wg"][:, ck, :],
                            start=(ck == 0),
                            stop=(ck == 1),
                        )
                    gt = stage.tile([128, HC], f32, tag="gtmp")
                    nc.vector.tensor_add(gt, ps[:, 0, :HC, 0], bg_bc)
                    nc.scalar.activation(Gn[:, qt, :], gt, AF.Sigmoid)

            # ---- main attention, software-pipelined on the PE:
            # QK(h) ... PV(h-1), so PV never waits on the bias-add/exp. ----
            def qk_block(h):
                j, rows = h // 4, hsl(h)
                tt = twp.tile([128, H, 2, 512], bf16, tag="t")
                et = ewp.tile([128, H, 2, 512], bf16, tag="et")
                Bt, B2t = bs_tiles[h]
                nc.vector.tensor_tensor(Bt[:, 4:8], Bt[:, 4:8], B2t, ALU.add)
                for pr in range(8):
                    ps = psp.tile([128, 2, 512, 1], f32, tag="scores", bufs=2)
                    for s in range(2):
                        kt = pr * 2 + s
                        nc.tensor.matmul(
                            ps[:, s, :, 0],
                            KT[rows, j, kt * 128:(kt + 1) * 128],
                            QT[rows, j, :],
                            start=True,
                            stop=True,
                            tile_position=((h % 4) * 32, 0),
                        )
                    nc.vector.tensor_tensor(
                        tt[:, pr], ps[:, :, :, 0], Bt[:, pr], ALU.add
                    )
                nc.scalar.activation(et, tt, AF.Exp)
                return et

            def pv_block(h, et):
                hcol = h * 33
                o_ps = psp.tile([33, QS, 1], f32, tag="o_acc", bufs=2)
                for kt in range(KTILES):
                    nc.tensor.matmul(
                        o_ps[:, :, 0],
                        Vn[:, kt, hcol:hcol + 33],
                        et[:, kt // 2, kt % 2, :],
                        start=(kt == 0),
                        stop=(kt == KTILES - 1),
                    )
                nc.vector.tensor_copy(oTs[:, h, :], o_ps[:, :, 0])
                for qt in range(4):
                    on_ps = psp.tile([128, 33, 1], f32, tag="onat", bufs=1)
                    nc.tensor.transpose(
                        on_ps[:, :, 0],
                        oTs[:, h, qt * 128:(qt + 1) * 128],
                        identf[:33, :33],
                    )
                    rinv = stage.tile([128, 1], f32, tag="rinv")
                    nc.vector.reciprocal(rinv, on_ps[:, 32:33, 0])
                    nc.vector.tensor_scalar_mul(
                        O_all[:, qt, h * 32:(h + 1) * 32], on_ps[:, :32, 0], rinv
                    )

            proj_kq(0)
            proj_g()
            prev = (0, qk_block(0))
            proj_v()
            proj_kq(1)
            for h in range(1, H):
                et = qk_block(h)
                pv_block(*prev)
                prev = (h, et)
            pv_block(*prev)

            # ---- gating + output projection ----
            for qt in range(4):
                og = stage.tile([128, HC], bf16, tag="og")
                nc.gpsimd.tensor_tensor(
                    og, O_all[:, qt, :], Gn[:, qt, :], ALU.mult
                )
                ogt_ps = psp.tile([128, 2, 128], bf16, tag="et_ps", bufs=1)
                for hcc in range(2):
                    nc.tensor.transpose(
                        ogt_ps[:, hcc, :], og[:, hcc * 128:(hcc + 1) * 128], ident
                    )
                ogt = stage.tile([128, 2, 128], bf16, tag="ogt")
                nc.vector.tensor_copy(ogt, ogt_ps)
                f_ps = psp.tile([128, 2, 512, 1], f32, tag="scores", bufs=2)
                for hcc in range(2):
                    nc.tensor.matmul(
                        f_ps[:, 0, :CO, 0],
                        ogt[:, hcc, :],
                        wbf["Wo"][:, hcc, :],
                        start=(hcc == 0),
                        stop=(hcc == 1),
                    )
                o_sb = stage.tile([128, CO], f32, tag="o_out")
                nc.vector.tensor_add(o_sb, f_ps[:, 0, :CO, 0], bo_bc)
                nc.scalar.dma_start(out=out[qt * 128:(qt + 1) * 128, :], in_=o_sb)

    nc.compile()
    return nc


def _get_nc():
    if "nc" not in _CACHED:
        _CACHED["nc"] = _build()
    return _CACHED["nc"]


def kernel(**inputs):
    from concourse.bass_utils import run_bass_kernel_spmd

    import ml_dtypes

    bf = ml_dtypes.bfloat16
    nc = _get_nc()
    inp = {k: np.asarray(v, dtype=np.float32) for k, v in inputs.items()}
    wq_b = (inp["Wq"] * SCALE).astype(bf)
    wk_b = inp["Wk"].astype(bf)
    wv_b = inp["Wv"].astype(bf)
    wg_b = inp["Wg"].astype(bf)
    wo_b = inp["Wo"].astype(bf)
    # on-chip k order is m = kt*128 + p with k = (kt//2)*256 + 2p + (kt%2);
    # permute kvxT columns so every on-chip consumer stays contiguous
    m = np.arange(K)
    kmap = (m // 256) * 256 + 2 * (m % 128) + (m // 128) % 2
    in_maps = []
    for c in range(N_CORES):
        b, qi = c // 4, c % 4
        q0 = qi * QS
        in_maps.append({
            "qxT": np.ascontiguousarray(inp["q_x"][b, q0:q0 + QS, :].T).astype(bf),
            "kvxT": np.ascontiguousarray(inp["kv_x"][b].T[:, kmap]).astype(bf),
            "b1": np.ascontiguousarray(
                inp["bias1"][b, :, q0:q0 + QS, :].transpose(0, 2, 1)
            ).astype(bf).reshape(H, 8, 128, 2, QS),
            "b2": np.ascontiguousarray(
                inp["bias2"][b, :, q0:q0 + QS, :].transpose(0, 2, 1)
            ).astype(bf).reshape(H, 8, 128, 2, QS),
            "Wq": wq_b, "Wk": wk_b, "Wv": wv_b, "Wg": wg_b,
            "bg": inp["bg"], "Wo": wo_b, "bo": inp["bo"],
        })
    res = run_bass_kernel_spmd(nc, in_maps, core_ids=list(range(N_CORES)))
    outa = np.empty((B, Q, CO), np.float32)
    for c in range(N_CORES):
        b, qi = c // 4, c % 4
        outa[b, qi * QS:(qi + 1) * QS, :] = res.results[c]["out"]
    return outa
